# revision 23
# baseline (speedup 1.0000x reference)
"""Trainium2 kernel for CFA-style KNN retrieval scoring.

Computes, for each row of phi [B*HW, C]:
  d_m = sqrt(|phi|^2 + |c_m|^2 - 2 phi.c_m)  over M bank entries,
  top-3 smallest distances d0<=d1<=d2,
  score = d0 / (1 + exp(d0-d1) + exp(d0-d2))      (= softmin weight * d0)

Strategy (8 NeuronCores, data-parallel over rows):
 - shard rows (B*HW = 50176) into 8 contiguous chunks of 6272 rows
 - bf16 matmul on TensorE (fp32 PSUM accumulate); the -0.5*|c_m|^2 term is
   folded into the accumulation via a K=2 ones-matmul with a bf16 hi/lo
   split of the centers row (keeps its error ~1e-3 instead of bf16 ulp ~4)
 - selection runs on psum = phi.c - 0.5|c|^2 directly (|phi|^2 is constant
   per row, monotone under sqrt -> does not affect the ranking); DVE `max`
   (top-8) per 448-wide PSUM tile, then a second `max` over 56 candidates
 - |phi|^2 computed in fp32 on ScalarE (activation Square + accum)
 - final softmin math on 3 values/row at the end, batched over all tiles
"""

import numpy as np

B, HW, C, M = 16, 3136, 1792, 3136
NCORES = 8
ROWS = B * HW // NCORES     # 6272 rows per core
P = 128                     # partitions
NT = ROWS // P              # 49 row-tiles per core; row = p*NT + t
KC = C // P                 # 14 contraction chunks
MT = 448                    # matmul moving free size (one PSUM bank)
NMT = M // MT               # 7 m-tiles

_CACHE = {}


def _build_program(nt=NT, reps=1):
    import contextlib
    import concourse.mybir as mybir
    from concourse import bacc
    from concourse.tile import TileContext
    from concourse.masks import make_identity

    f32 = mybir.dt.float32
    bf16 = mybir.dt.bfloat16
    rows = P * nt

    nc = bacc.Bacc("TRN2", target_bir_lowering=False, debug=False)
    phi = nc.dram_tensor("phi", [rows, C], f32, kind="ExternalInput")
    cbank = nc.dram_tensor("cbank", [C, M], bf16, kind="ExternalInput")
    cc2 = nc.dram_tensor("cc2", [2, M], bf16, kind="ExternalInput")
    out = nc.dram_tensor("out", [rows, 1], f32, kind="ExternalOutput")

    phi_r = phi[:, :].rearrange("(p t) c -> p t c", t=nt)
    out_r = out[:, :].rearrange("(p t) o -> p (t o)", t=nt)

    with TileContext(nc) as tc:
        with (
            tc.tile_pool(name="const", bufs=1) as const_pool,
            tc.tile_pool(name="cb", bufs=1) as cb_pool,
            tc.tile_pool(name="stage", bufs=3) as stage_pool,
            tc.tile_pool(name="bfp", bufs=2) as bfp_pool,
            tc.tile_pool(name="sq", bufs=2) as sq_pool,
            tc.tile_pool(name="lhsT", bufs=2) as lhsT_pool,
            tc.tile_pool(name="cand", bufs=2) as cand_pool,
            tc.tile_pool(name="tp", bufs=2, space="PSUM") as tpsum_pool,
            tc.tile_pool(name="mm", bufs=3, space="PSUM") as mm_pool,
            tc.tile_pool(name="acc", bufs=1) as acc_pool,
            tc.tile_pool(name="fin", bufs=1) as fin_pool,
        ):
            ident = const_pool.tile([P, P], bf16)
            make_identity(nc, ident[:])
            ones2 = const_pool.tile([2, P], bf16)
            nc.vector.memset(ones2[:], 1.0)
            cc2_sb = const_pool.tile([2, M], bf16)
            nc.sync.dma_start(cc2_sb[:], cc2[:, :])

            cbt = []
            for k in range(KC):
                ct = cb_pool.tile([P, M], bf16, tag=f"cb{k}")
                nc.sync.dma_start(ct[:], cbank[k * P:(k + 1) * P, :])
                cbt.append(ct)

            feat = acc_pool.tile([P, nt], f32)
            allv = acc_pool.tile([P, nt * 8], f32)

            def body():
                for t in range(nt):
                    stg = stage_pool.tile([P, C], f32)
                    nc.sync.dma_start(stg[:], phi_r[:, t, :])
                    phib = bfp_pool.tile([P, C], bf16)
                    nc.scalar.copy(phib[:], stg[:])
                    sqt = sq_pool.tile([P, C], bf16)
                    nc.scalar.activation(
                        sqt[:], stg[:], mybir.ActivationFunctionType.Square,
                        accum_out=feat[:, t:t + 1],
                    )

                    tp = tpsum_pool.tile([P, KC * P], bf16)
                    for k in range(KC):
                        nc.tensor.transpose(
                            tp[:, k * P:(k + 1) * P], phib[:, k * P:(k + 1) * P],
                            ident[:],
                        )
                    lt = lhsT_pool.tile([P, KC * P], bf16)
                    nc.vector.tensor_copy(lt[:], tp[:])

                    cand = cand_pool.tile([P, NMT * 8], f32)
                    for j in range(NMT):
                        ps = mm_pool.tile([P, MT], f32)
                        for k in range(KC):
                            nc.tensor.matmul(
                                ps[:],
                                lhsT=lt[:, k * P:(k + 1) * P],
                                rhs=cbt[k][:, j * MT:(j + 1) * MT],
                                start=(k == 0), stop=False,
                            )
                        nc.tensor.matmul(
                            ps[:], lhsT=ones2[:],
                            rhs=cc2_sb[:, j * MT:(j + 1) * MT],
                            start=False, stop=True,
                        )
                        nc.vector.max(out=cand[:, j * 8:(j + 1) * 8], in_=ps[:])
                    nc.vector.max(out=allv[:, t * 8:(t + 1) * 8], in_=cand[:])

                # ---- final: d_i = sqrt(feat - 2*v_i), score = d0/(1+e^g1+e^g2)
                allv_r = allv[:].rearrange("p (t e) -> p e t", e=8)
                d2 = fin_pool.tile([P, 3 * nt], f32)
                for i in range(3):
                    tmp = fin_pool.tile([P, nt], f32, tag=f"tmp{i}")
                    nc.vector.tensor_scalar_mul(tmp[:], allv_r[:, i, :], 2.0)
                    nc.vector.tensor_sub(d2[:, i * nt:(i + 1) * nt], feat[:], tmp[:])
                d = fin_pool.tile([P, 3 * nt], f32)
                nc.scalar.sqrt(d[:], d2[:])
                g = fin_pool.tile([P, 2 * nt], f32)
                nc.vector.tensor_sub(g[:, :nt], d[:, :nt], d[:, nt:2 * nt])
                nc.vector.tensor_sub(g[:, nt:], d[:, :nt], d[:, 2 * nt:])
                e = fin_pool.tile([P, 2 * nt], f32)
                nc.scalar.activation(e[:], g[:], mybir.ActivationFunctionType.Exp)
                s = fin_pool.tile([P, nt], f32)
                nc.vector.tensor_add(s[:], e[:, :nt], e[:, nt:])
                nc.vector.tensor_scalar_add(s[:], s[:], 1.0)
                r = fin_pool.tile([P, nt], f32)
                nc.vector.reciprocal(r[:], s[:])
                sc = fin_pool.tile([P, nt], f32)
                nc.vector.tensor_mul(sc[:], d[:, :nt], r[:])
                nc.sync.dma_start(out_r, sc[:])

            if reps > 1:
                with tc.For_i(0, reps, 1):
                    body()
            else:
                body()

    return nc


def _build_program2(nt=NT, reps=1, korder="kinner", mm_bufs=3, do_max=True, do_feat=True, do_ltdma=True, lt_bufs=3, centers="mm", host_feat=False):
    """v2: phi arrives pre-transposed/bf16 from host (layout prep only);
    no PE transposes, no cast pass, no PSUM-evac copy.
    Row mapping: sbuf row-tile t holds phi rows {p*nt + t}; phit is laid out
    [nt*P, KC*P] with phit[t*128 + p', k*128 + n'] = phi[n'*nt + t, k*128 + p']
    so each tile's lhsT block is one contiguous 448KB DMA (3584B/partition),
    and the output DMA stays contiguous per partition."""
    import concourse.mybir as mybir
    from concourse import bacc
    from concourse.tile import TileContext

    f32 = mybir.dt.float32
    bf16 = mybir.dt.bfloat16
    rows = P * nt

    nc = bacc.Bacc("TRN2", target_bir_lowering=False, debug=False)
    phi = nc.dram_tensor("phi", [rows, C], f32, kind="ExternalInput")
    phit = nc.dram_tensor("phit", [rows, C], bf16, kind="ExternalInput")
    cbank = nc.dram_tensor("cbank", [C, M], bf16, kind="ExternalInput")
    cc2 = nc.dram_tensor("cc2", [2, M], bf16, kind="ExternalInput")
    ccf = (nc.dram_tensor("ccf", [P, M], f32, kind="ExternalInput")
           if centers != "mm" else None)
    featv = (nc.dram_tensor("featv", [P, nt], f32, kind="ExternalInput")
             if host_feat else None)
    out = nc.dram_tensor("out", [rows, 1], f32, kind="ExternalOutput")

    phi_r = phi[:, :].rearrange("(p t) c -> p t c", t=nt)      # feat loads
    phit_r = phit[:, :].rearrange("(t p) f -> t p f", p=P)     # lhsT loads
    out_r = out[:, :].rearrange("(p t) o -> p (t o)", t=nt)

    with TileContext(nc) as tc:
        with (
            tc.tile_pool(name="const", bufs=1) as const_pool,
            tc.tile_pool(name="cb", bufs=1) as cb_pool,
            tc.tile_pool(name="stage", bufs=3) as stage_pool,
            tc.tile_pool(name="sq", bufs=2) as sq_pool,
            tc.tile_pool(name="lhsT", bufs=lt_bufs) as lhsT_pool,
            tc.tile_pool(name="cand", bufs=2) as cand_pool,
            tc.tile_pool(name="mm", bufs=mm_bufs, space="PSUM") as mm_pool,
            tc.tile_pool(name="mmg", bufs=1, space="PSUM") as mmg_pool,
            tc.tile_pool(name="acc", bufs=1) as acc_pool,
            tc.tile_pool(name="fin", bufs=1) as fin_pool,
        ):
            ones2 = const_pool.tile([2, P], bf16)
            nc.vector.memset(ones2[:], 1.0)
            cc2_sb = const_pool.tile([2, M], bf16)
            nc.sync.dma_start(cc2_sb[:], cc2[:, :])
            ccf_sb = None
            if ccf is not None:
                ccf_sb = const_pool.tile([P, M], f32)
                nc.sync.dma_start(ccf_sb[:], ccf[:, :])

            cbt = []
            for k in range(KC):
                ct = cb_pool.tile([P, M], bf16, tag=f"cb{k}")
                nc.sync.dma_start(ct[:], cbank[k * P:(k + 1) * P, :])
                cbt.append(ct)

            feat = acc_pool.tile([P, nt], f32)
            allv = acc_pool.tile([P, nt * 8], f32)
            ltfix = None
            if not do_ltdma:
                ltfix = const_pool.tile([P, KC * P], bf16)
                nc.sync.dma_start(ltfix[:], phit_r[0])
            if not do_feat:
                nc.vector.memset(feat[:], 3584.0)
            if host_feat:
                nc.sync.dma_start(feat[:], featv[:, :])

            def body():
                for t in range(nt):
                    if do_feat and not host_feat:
                        stg = stage_pool.tile([P, C], f32)
                        nc.sync.dma_start(stg[:], phi_r[:, t, :])
                        sqt = sq_pool.tile([P, C], bf16)
                        nc.scalar.activation(
                            sqt[:], stg[:], mybir.ActivationFunctionType.Square,
                            accum_out=feat[:, t:t + 1],
                        )
                    if do_ltdma:
                        lt = lhsT_pool.tile([P, KC * P], bf16)
                        nc.sync.dma_start(lt[:], phit_r[t])
                    else:
                        lt = ltfix

                    cand = cand_pool.tile([P, NMT * 8], f32)
                    if korder == "kinner":
                        for j in range(NMT):
                            ps = mm_pool.tile([P, MT], f32)
                            for k in range(KC):
                                nc.tensor.matmul(
                                    ps[:],
                                    lhsT=lt[:, k * P:(k + 1) * P],
                                    rhs=cbt[k][:, j * MT:(j + 1) * MT],
                                    start=(k == 0),
                                    stop=(centers != "mm" and k == KC - 1),
                                )
                            if centers == "mm":
                                nc.tensor.matmul(
                                    ps[:], lhsT=ones2[:],
                                    rhs=cc2_sb[:, j * MT:(j + 1) * MT],
                                    start=False, stop=True,
                                )
                            else:
                                nc.vector.tensor_add(
                                    ps[:], ps[:],
                                    ccf_sb[:, j * MT:(j + 1) * MT],
                                )
                            if do_max:
                                nc.vector.max(out=cand[:, j * 8:(j + 1) * 8],
                                              in_=ps[:])
                    else:  # groups: lhsT constant across consecutive matmuls
                        for grp in ([0, 1, 2], [3, 4, 5, 6]):
                            pss = {j: mmg_pool.tile([P, MT], f32, tag=f"ps{j}",
                                                    name=f"ps{j}_{t}")
                                   for j in grp}
                            for k in range(KC):
                                for j in grp:
                                    nc.tensor.matmul(
                                        pss[j][:],
                                        lhsT=lt[:, k * P:(k + 1) * P],
                                        rhs=cbt[k][:, j * MT:(j + 1) * MT],
                                        start=(k == 0), stop=False,
                                    )
                            for j in grp:
                                nc.tensor.matmul(
                                    pss[j][:], lhsT=ones2[:],
                                    rhs=cc2_sb[:, j * MT:(j + 1) * MT],
                                    start=False, stop=True,
                                )
                            for j in grp:
                                nc.vector.max(out=cand[:, j * 8:(j + 1) * 8],
                                              in_=pss[j][:])
                    if do_max:
                        nc.vector.max(out=allv[:, t * 8:(t + 1) * 8], in_=cand[:])

                if not do_max:
                    nc.sync.dma_start(out_r, feat[:])
                    return
                # ---- final softmin math (same as v1)
                allv_r = allv[:].rearrange("p (t e) -> p e t", e=8)
                d2 = fin_pool.tile([P, 3 * nt], f32)
                for i in range(3):
                    tmp = fin_pool.tile([P, nt], f32, tag=f"tmp{i}")
                    nc.vector.tensor_scalar_mul(tmp[:], allv_r[:, i, :], 2.0)
                    nc.vector.tensor_sub(d2[:, i * nt:(i + 1) * nt], feat[:], tmp[:])
                d = fin_pool.tile([P, 3 * nt], f32)
                nc.scalar.sqrt(d[:], d2[:])
                g = fin_pool.tile([P, 2 * nt], f32)
                nc.vector.tensor_sub(g[:, :nt], d[:, :nt], d[:, nt:2 * nt])
                nc.vector.tensor_sub(g[:, nt:], d[:, :nt], d[:, 2 * nt:])
                e = fin_pool.tile([P, 2 * nt], f32)
                nc.scalar.activation(e[:], g[:], mybir.ActivationFunctionType.Exp)
                s = fin_pool.tile([P, nt], f32)
                nc.vector.tensor_add(s[:], e[:, :nt], e[:, nt:])
                nc.vector.tensor_scalar_add(s[:], s[:], 1.0)
                r = fin_pool.tile([P, nt], f32)
                nc.vector.reciprocal(r[:], s[:])
                sc = fin_pool.tile([P, nt], f32)
                nc.vector.tensor_mul(sc[:], d[:, :nt], r[:])
                nc.sync.dma_start(out_r, sc[:])

            if reps > 1:
                with tc.For_i(0, reps, 1):
                    body()
            else:
                body()

    return nc


def _host_prep_phit(phi_core, nt=NT):
    """[rows, C] f32 -> [nt*P, KC*P] bf16, laid out so lhsT tile t is one
    contiguous 448KB block: phit[t*128 + p', k*128 + n'] = phi[t*128 + n', k*128 + p']."""
    import ml_dtypes
    # tile t, sbuf partition p' (= contraction c_local), free n' (= within-tile
    # row index); within-tile row n' maps to phi row n'*nt + t (v1 mapping).
    x = phi_core.reshape(P, nt, KC, P).transpose(1, 3, 2, 0)   # [t, p', k, n']
    return np.ascontiguousarray(x.reshape(nt * P, KC * P).astype(ml_dtypes.bfloat16))


def _host_prep(C_bank):
    import ml_dtypes
    bf = ml_dtypes.bfloat16
    cb_bf = np.ascontiguousarray(C_bank.astype(bf))
    row = -0.5 * (C_bank.astype(np.float64) ** 2).sum(0)
    chi = row.astype(np.float32).astype(bf)
    clo = (row - chi.astype(np.float64)).astype(np.float32).astype(bf)
    cc2 = np.ascontiguousarray(np.stack([chi, clo]))
    ccf = np.ascontiguousarray(
        np.broadcast_to(row.astype(np.float32), (P, C_bank.shape[1])))
    return cb_bf, cc2, ccf


def kernel(phi_p: np.ndarray, C_bank: np.ndarray) -> np.ndarray:
    from concourse.bass_utils import run_bass_kernel_spmd

    if "nc" not in _CACHE:
        nc = _build_program2(mm_bufs=6)
        nc.finalize()
        _CACHE["nc"] = nc
    nc = _CACHE["nc"]

    phi_p = np.asarray(phi_p, dtype=np.float32)
    C_bank = np.asarray(C_bank, dtype=np.float32)
    cb_bf, cc2, ccf = _host_prep(C_bank)
    phi2 = np.ascontiguousarray(phi_p.reshape(B * HW, C))
    in_maps = [
        {"phi": phi2[k * ROWS:(k + 1) * ROWS],
         "phit": _host_prep_phit(phi2[k * ROWS:(k + 1) * ROWS]),
         "cbank": cb_bf, "cc2": cc2}
        for k in range(NCORES)
    ]
    res = None
    for attempt in range(3):
        try:
            res = run_bass_kernel_spmd(nc, in_maps, list(range(NCORES)))
            break
        except Exception:
            # transient NRT device errors have been observed; reset the jax
            # backend connection and retry
            if attempt == 2:
                raise
            import time as _time
            _time.sleep(5)
            try:
                import jax
                jax.clear_caches()
                jax.extend.backend.clear_backends()
            except Exception:
                pass
    out = np.concatenate([res.results[k]["out"] for k in range(NCORES)], axis=0)
    return out.reshape(B, HW, 1)
